# revision 37
# baseline (speedup 1.0000x reference)
"""KNN (k=16, 10 classes) on 8 Trainium2 NeuronCores via Bass/Tile.

Distributed ANN: shard X_train across 8 cores; each core scores its shard
against all 4096 queries and returns per-class-window top-8 candidates;
host merges to global top-16 and votes.

Scores v[q, j] = 2<t_q, x_j> - ||x_j||^2 are computed in 3 fp16 matmul
passes per 512-col PSUM bank (fp16 streams at 1 cycle/row vs 4 for fp32,
and 2-byte weights pipeline their LDWEIGHTS):
  pass1: q_h . x_h           (all 128 dims; fp16 products are exact in
  pass2: q_l . x_h            fp32 PSUM accumulation; same moving tensor
                              as pass1 -> x_h is loaded from HBM once)
  pass3: q_h . x_l [0:126]
         + rows126/127 borrowed to add -||x||^2 hi/lo (fp16)
where q = 2*X_test = q_h + q_l and x = x_h + x_l are fp16 hi/lo splits.
Dropped terms (q_l.x_l, dim-126/127 x_l cross terms) give ~6e-4 rms
score error vs fp64 (0-1 vote flips; the merge gate needs <~1e-3, which
rules out 2-pass fp16 (6e-3), 1-pass fp32r (3.3e-3 measured on HW) and
fp8-DoubleRow hybrids (e4m3 dynamic range can't hold the correction
terms), so 3 passes at 1 col/cycle is the PE optimum).
The last PSUM bank is partial: matmuls stream only the ~12503 content
columns, not the 512-padded 12800.

Train layout per core: for each class c exactly s_c rows (identical s_c
on all cores; shortfall padded with dummy rows scoring -6e4) so the SPMD
program's class-window scan offsets are the same on every core, and a
candidate's position identifies its label.  DVE max8 scans each class
window (split at PSUM ring wraps and at matmul-group boundaries so
banks release to the PE promptly) -> top-8 values per (query, piece);
candidates accumulate in SBUF, one bulk DMA out, host merges 8 cores x
~18 pieces x 8 values per query -> top-16 -> majority vote (argmax
ties -> smallest label, matching the reference).
"""

import numpy as np

NCORES = 8
D = 128
QTILE = 128
NUM_CLASSES = 10
BANK = 512
RING_BANKS = 8
RING = BANK * RING_BANKS  # 4096
PAD_XXH = np.float16(-60000.0)

_compiled_cache = {}


def _class_sizes(y):
    """Per-core per-class allocation s_c (identical across cores)."""
    n = np.bincount(y, minlength=NUM_CLASSES)
    s = [(int(nc) + NCORES - 1) // NCORES for nc in n]
    # adjust so every window boundary b has b%512 in {0} U [8, 504]
    # (guarantees every ring-split scan piece is >= 8 wide for max8)
    off = 0
    out = []
    for c in range(NUM_CLASSES):
        sc = max(s[c], 8)
        while True:
            r = (off + sc) % BANK
            if r == 0 or 8 <= r <= 504:
                break
            sc += 1
        out.append(sc)
        off += sc
    return out


def _layout(y):
    s = _class_sizes(y)
    tot = sum(s)
    Lp = ((tot + BANK - 1) // BANK) * BANK
    offs = np.concatenate([[0], np.cumsum(s)])  # class windows [offs[c], offs[c+1])
    return s, offs, Lp


GROUP = 4  # banks per matmul group (2*GROUP <= RING_BANKS)


def _tail_gstarts(nbanks):
    """Matmul group starts for the LAST qtile: normal GROUP-wide groups,
    but the final full group is split 2+2 (plus the partial last bank) so
    the end-of-kernel scans overlap the matmul stream instead of trailing
    it."""
    t0 = ((nbanks - 1) // GROUP) * GROUP  # start of the partial tail group
    gs = list(range(0, max(t0 - GROUP, 0), GROUP))
    if t0 >= GROUP:
        gs += list(range(t0 - GROUP, t0))  # 1-bank groups for the last few
    gs += [t0]
    return gs


def _pieces_for_qtile(qt, offs, Lp, nbanks, last=False):
    """Scan pieces for one query tile: class windows split at ring wraps
    AND at matmul-group boundaries (so each piece's max8 only waits on its
    own group's last pass, releasing PSUM banks to the PE sooner).  The
    last qtile also splits at its finer tail-group boundaries.

    Tail padding past offs[-1] is never scanned (its scores are garbage-
    free anyway, but skipping it saves DVE cycles).

    Returns list of (col_off, length, class)."""
    x0 = (-nbanks * qt * BANK) % RING
    splits = set(range(x0, Lp + 1, RING)) | set(range(0, Lp + 1, GROUP * BANK))
    if last:
        splits |= {g * BANK for g in _tail_gstarts(nbanks)}
    pieces = []
    for c in range(NUM_CLASSES):
        o, e = int(offs[c]), int(offs[c + 1])
        cuts = sorted([o, e] + [x for x in splits if o < x < e])
        for a, b in zip(cuts[:-1], cuts[1:]):
            pieces.append((a, b - a, c))
    return pieces


def _build_program(NQ, Lp, offs):
    import concourse.bacc as bacc
    import concourse.tile as tile
    import concourse.mybir as mybir

    nbanks = Lp // BANK
    ctot = int(offs[-1])  # real content columns; the rest is tail padding
    nqt = NQ // QTILE

    def bank_w(b):
        # partial last bank: matmuls stream only the content columns
        return min(BANK, ctot - b * BANK)

    all_pieces = [_pieces_for_qtile(qt, offs, Lp, nbanks, last=(qt == nqt - 1))
                  for qt in range(nqt)]
    slot_base = np.cumsum([0] + [len(p) for p in all_pieces])
    STOT = int(slot_base[-1])

    f16 = mybir.dt.float16
    nc = bacc.Bacc("TRN2", target_bir_lowering=False, debug=False)
    qh = nc.dram_tensor("qh", [D, NQ], f16, kind="ExternalInput")
    s2 = nc.dram_tensor("s2", [D, NQ], f16, kind="ExternalInput")
    s3 = nc.dram_tensor("s3", [D, NQ], f16, kind="ExternalInput")
    # ma = x_h, shared by pass1 (qh) and pass2 (q_l): loaded from HBM once.
    # mb = [x_l rows 0-125; xx_h; xx_l] for pass3.
    ma = nc.dram_tensor("ma", [D, Lp], f16, kind="ExternalInput")
    mb = nc.dram_tensor("mb", [D, Lp], f16, kind="ExternalInput")
    out_cand = nc.dram_tensor("out_cand", [QTILE, STOT * 8], mybir.dt.float32,
                              kind="ExternalOutput")

    with tile.TileContext(nc) as tc:
        with tc.tile_pool(name="res", bufs=1) as res, \
             tc.tile_pool(name="ring", bufs=1, space="PSUM") as ringp:
            qh_t = res.tile([D, NQ], f16)
            s2_t = res.tile([D, NQ], f16)
            s3_t = res.tile([D, NQ], f16)
            ma_t = res.tile([D, Lp], f16)
            mb_t = res.tile([D, Lp], f16)
            cand_t = res.tile([QTILE, STOT * 8], mybir.dt.float32)
            # warm the PE's HAM clock gate while input DMAs are in flight:
            # junk matmuls on a zeroed SBUF tile run as soon as the (gpsimd)
            # memset lands, keeping the PE at full clock by the time real
            # work starts (values are overwritten by start=True).
            ring = ringp.tile([QTILE, RING], mybir.dt.float32)
            junk = res.tile([D, BANK], f16)
            nc.gpsimd.memset(junk[:], 0.0)

            # load order matters: get exactly what qtile 0 group 0 needs
            # first (qh slice + first 4 ma banks), then the rest in
            # consumption order, so real matmuls can start ~1.7us after
            # DMA kickoff instead of waiting for full first chunks.
            QSPLIT = 8
            qs = NQ // QSPLIT
            g0w = GROUP * BANK

            def qsl_i(i):
                return slice(i * qs, (i + 1) * qs)

            nc.sync.dma_start(out=qh_t[:, 0:QTILE], in_=qh[:, 0:QTILE])
            nc.sync.dma_start(out=ma_t[:, 0:g0w], in_=ma[:, 0:g0w])
            nc.sync.dma_start(out=s2_t[:, 0:QTILE], in_=s2[:, 0:QTILE])
            nc.sync.dma_start(out=mb_t[:, 0:g0w], in_=mb[:, 0:g0w])
            nc.sync.dma_start(out=s3_t[:, 0:QTILE], in_=s3[:, 0:QTILE])
            # all m chunks before the remaining q slices: qtile 0 consumes
            # every m column in its first ~16us, while qtile 1+ only needs
            # later q slices after that.
            mcuts = [g0w]
            while mcuts[-1] < ctot:
                mcuts.append(min(mcuts[-1] + 2048, ctot))
            for a, b in zip(mcuts[:-1], mcuts[1:]):
                if a < b:
                    for t, d in ((ma_t, ma), (mb_t, mb)):
                        nc.sync.dma_start(out=t[:, a:b], in_=d[:, a:b])
            for t, d in ((qh_t, qh), (s2_t, s2), (s3_t, s3)):
                nc.sync.dma_start(out=t[:, QTILE:qs], in_=d[:, QTILE:qs])
            for i in range(1, QSPLIT):
                for t, d in ((qh_t, qh), (s2_t, s2), (s3_t, s3)):
                    nc.sync.dma_start(out=t[:, qsl_i(i)], in_=d[:, qsl_i(i)])

            # 9 junk matmuls: keeps the PE busy from preamble end until
            # the first real inputs land (~12us), so the HAM SHORT window
            # trips to full clock with no idle gap.
            for w in range(9):
                nc.tensor.matmul(ring[:, (w % RING_BANKS) * BANK:
                                       (w % RING_BANKS) * BANK + BANK],
                                 lhsT=junk[:, 0:QTILE], rhs=junk[:],
                                 start=True, stop=True)

            for qt in range(nqt):
                qsl = slice(qt * QTILE, (qt + 1) * QTILE)
                pieces = all_pieces[qt]
                base = int(slot_base[qt])
                emitted = 0

                def emit_scans(done_cols, emitted):
                    while emitted < len(pieces):
                        o, ln, _c = pieces[emitted]
                        if o + ln > done_cols:
                            break
                        ro = (qt * nbanks * BANK + o) % RING
                        slot = (base + emitted) * 8
                        nc.vector.max(out=cand_t[:, slot:slot + 8],
                                      in_=ring[:, ro:ro + ln])
                        emitted += 1
                    return emitted

                if qt == nqt - 1:
                    gstarts = _tail_gstarts(nbanks)
                else:
                    gstarts = list(range(0, nbanks, GROUP))
                gends = gstarts[1:] + [nbanks]
                for g0, g1 in zip(gstarts, gends):
                    banks = range(g0, g1)

                    def rsl(b):
                        s = ((qt * nbanks + b) % RING_BANKS) * BANK
                        return slice(s, s + bank_w(b))

                    def csl(b):
                        return slice(b * BANK, b * BANK + bank_w(b))

                    # per-bank pass order: each bank's scores complete 2
                    # passes earlier than pass-major order, so its max8
                    # starts sooner and the PSUM bank frees sooner; pieces
                    # ending mid-group (class-window boundaries) emit
                    # right after their bank's last pass.  Exception: the
                    # first two groups of qtile 0 run pass-major, deferring
                    # their mb (pass3) needs ~1.7us while its DMA lands.
                    if qt == 0 and g0 < 2 * GROUP:
                        for b in banks:
                            nc.tensor.matmul(ring[:, rsl(b)], lhsT=qh_t[:, qsl],
                                             rhs=ma_t[:, csl(b)],
                                             start=True, stop=False)
                        for b in banks:
                            nc.tensor.matmul(ring[:, rsl(b)], lhsT=s2_t[:, qsl],
                                             rhs=ma_t[:, csl(b)],
                                             start=False, stop=False)
                        for b in banks:
                            nc.tensor.matmul(ring[:, rsl(b)], lhsT=s3_t[:, qsl],
                                             rhs=mb_t[:, csl(b)],
                                             start=False, stop=True)
                            emitted = emit_scans((b + 1) * BANK, emitted)
                    else:
                        for b in banks:
                            nc.tensor.matmul(ring[:, rsl(b)], lhsT=qh_t[:, qsl],
                                             rhs=ma_t[:, csl(b)],
                                             start=True, stop=False)
                            nc.tensor.matmul(ring[:, rsl(b)], lhsT=s2_t[:, qsl],
                                             rhs=ma_t[:, csl(b)],
                                             start=False, stop=False)
                            nc.tensor.matmul(ring[:, rsl(b)], lhsT=s3_t[:, qsl],
                                             rhs=mb_t[:, csl(b)],
                                             start=False, stop=True)
                            emitted = emit_scans((b + 1) * BANK, emitted)

                # stream candidates out in tapered batches: big batches
                # early, single qtiles at the end so the final flush (and
                # its descriptor-generation latency) is tiny
                flush = {7: 0, 15: 8, 23: 16, 27: 24, 29: 28, 30: 30, 31: 31}
                if qt in flush:
                    lo8 = int(slot_base[flush[qt]]) * 8
                    hi8 = int(slot_base[qt + 1]) * 8
                    nc.sync.dma_start(out=out_cand[:, lo8:hi8],
                                      in_=cand_t[:, lo8:hi8])

    nc.compile()
    return nc, all_pieces, slot_base


def _get_program(NQ, Lp, offs):
    key = (NQ, Lp, tuple(int(o) for o in offs))
    if key not in _compiled_cache:
        _compiled_cache[key] = _build_program(NQ, Lp, offs)
    return _compiled_cache[key]


def prepare(X_train, y_train, X_test):
    X_train = np.ascontiguousarray(np.asarray(X_train, dtype=np.float32))
    X_test = np.ascontiguousarray(np.asarray(X_test, dtype=np.float32))
    y = np.asarray(y_train).astype(np.int64)
    N, Dd = X_train.shape
    NQ = X_test.shape[0]
    assert Dd == D

    s, offs, Lp = _layout(y)

    order = np.argsort(y, kind="stable")
    n = np.bincount(y, minlength=NUM_CLASSES)
    cstart = np.concatenate([[0], np.cumsum(n)])

    xx = np.einsum("ij,ij->i", X_train.astype(np.float64),
                   X_train.astype(np.float64)).astype(np.float32)

    # per-core fp16 hi/lo train splits, class-contiguous layout
    xh = np.zeros((NCORES, D, Lp), np.float16)
    xl = np.zeros((NCORES, D, Lp), np.float16)
    xxh = np.full((NCORES, Lp), PAD_XXH, np.float16)
    xxl = np.zeros((NCORES, Lp), np.float16)
    for c in range(NUM_CLASSES):
        rows = order[cstart[c]:cstart[c + 1]]
        sc = s[c]
        o = int(offs[c])
        for k in range(NCORES):
            sub = rows[k * sc:(k + 1) * sc]
            m = len(sub)
            if m:
                xt32 = X_train[sub].T  # [D, m] fp32
                h = xt32.astype(np.float16)
                xh[k, :, o:o + m] = h
                xl[k, :, o:o + m] = (xt32 - h.astype(np.float32)).astype(np.float16)
                nx = -xx[sub]
                nh = nx.astype(np.float16)
                xxh[k, o:o + m] = nh
                xxl[k, o:o + m] = (nx - nh.astype(np.float32)).astype(np.float16)

    # moving tensors: ma = xh (shared by pass1+pass2);
    # mb = xl with rows 126/127 := xxh/xxl
    mb = xl.copy()
    mb[:, D - 2, :] = xxh
    mb[:, D - 1, :] = xxl

    # stationary tensors: qh = fp16(2*X_test)^T (pass1); s2 = q_l rows
    # (pass2, full 128 dims); s3 = [q_h rows 0-125; 1; 1] (pass3)
    q32 = (2.0 * X_test).T.astype(np.float32)          # [D, NQ]
    qh = q32.astype(np.float16)
    ql = (q32 - qh.astype(np.float32)).astype(np.float16)
    s3m = qh.copy()
    s3m[D - 2, :] = np.float16(1.0)
    s3m[D - 1, :] = np.float16(1.0)

    nc, all_pieces, slot_base = _get_program(NQ, Lp, offs)

    in_maps = [{"qh": qh, "s2": ql, "s3": s3m,
                "ma": np.ascontiguousarray(xh[k]),
                "mb": np.ascontiguousarray(mb[k])}
               for k in range(NCORES)]

    slot_labels = np.concatenate(
        [np.repeat([c for (_o, _l, c) in pieces], 8) for pieces in all_pieces])
    meta = (all_pieces, slot_base, slot_labels, NQ)
    return nc, in_maps, meta


def merge(results, meta):
    all_pieces, slot_base, slot_labels, NQ = meta
    nqt = NQ // QTILE
    K = 16
    vals = np.stack([results[k]["out_cand"] for k in range(NCORES)], axis=0)
    preds = np.empty(NQ, np.int64)
    for qt in range(nqt):
        lo = int(slot_base[qt]) * 8
        hi = int(slot_base[qt + 1]) * 8
        v = vals[:, :, lo:hi]                        # [NCORES, QTILE, W]
        lab = slot_labels[lo:hi]
        v = np.moveaxis(v, 1, 0).reshape(QTILE, -1)  # [QTILE, NCORES*W]
        labs = np.tile(lab, NCORES)
        sel = np.argpartition(-v, K - 1, axis=1)[:, :K]
        top_lab = labs[sel]
        counts = np.zeros((QTILE, NUM_CLASSES), np.int64)
        for c in range(NUM_CLASSES):
            counts[:, c] = (top_lab == c).sum(1)
        preds[qt * QTILE:(qt + 1) * QTILE] = counts.argmax(1)
    return preds.astype(np.int64)


def kernel(X_train, y_train, X_test):
    from concourse.bass_utils import run_bass_kernel_spmd
    nc, in_maps, meta = prepare(X_train, y_train, X_test)
    res = run_bass_kernel_spmd(nc, in_maps, core_ids=list(range(NCORES)))
    return merge(res.results, meta)



# revision 39
# speedup vs baseline: 1.0014x; 1.0014x over previous
"""KNN (k=16, 10 classes) on 8 Trainium2 NeuronCores via Bass/Tile.

Distributed ANN: shard X_train across 8 cores; each core scores its shard
against all 4096 queries and returns per-class-window top-8 candidates;
host merges to global top-16 and votes.

Scores v[q, j] = 2<t_q, x_j> - ||x_j||^2 are computed in 3 fp16 matmul
passes per 512-col PSUM bank (fp16 streams at 1 cycle/row vs 4 for fp32,
and 2-byte weights pipeline their LDWEIGHTS):
  pass1: q_h . x_h           (all 128 dims; fp16 products are exact in
  pass2: q_l . x_h            fp32 PSUM accumulation; same moving tensor
                              as pass1 -> x_h is loaded from HBM once)
  pass3: q_h . x_l [0:126]
         + rows126/127 borrowed to add -||x||^2 hi/lo (fp16)
where q = 2*X_test = q_h + q_l and x = x_h + x_l are fp16 hi/lo splits.
Dropped terms (q_l.x_l, dim-126/127 x_l cross terms) give ~6e-4 rms
score error vs fp64 (0-1 vote flips; the merge gate needs <~1e-3, which
rules out 2-pass fp16 (6e-3), 1-pass fp32r (3.3e-3 measured on HW) and
fp8-DoubleRow hybrids (e4m3 dynamic range can't hold the correction
terms), so 3 passes at 1 col/cycle is the PE optimum).
The last PSUM bank is partial: matmuls stream only the ~12503 content
columns, not the 512-padded 12800.

Train layout per core: for each class c exactly s_c rows (identical s_c
on all cores; shortfall padded with dummy rows scoring -6e4) so the SPMD
program's class-window scan offsets are the same on every core, and a
candidate's position identifies its label.  DVE max8 scans each class
window (split at PSUM ring wraps and at matmul-group boundaries so
banks release to the PE promptly) -> top-8 values per (query, piece);
candidates accumulate in SBUF, one bulk DMA out, host merges 8 cores x
~18 pieces x 8 values per query -> top-16 -> majority vote (argmax
ties -> smallest label, matching the reference).
"""

import numpy as np

NCORES = 8
D = 128
QTILE = 128
NUM_CLASSES = 10
BANK = 512
RING_BANKS = 8
RING = BANK * RING_BANKS  # 4096
PAD_XXH = np.float16(-60000.0)

_compiled_cache = {}


def _class_sizes(y):
    """Per-core per-class allocation s_c (identical across cores)."""
    n = np.bincount(y, minlength=NUM_CLASSES)
    s = [(int(nc) + NCORES - 1) // NCORES for nc in n]
    # adjust so every window boundary b has b%512 in {0} U [8, 504]
    # (guarantees every ring-split scan piece is >= 8 wide for max8)
    off = 0
    out = []
    for c in range(NUM_CLASSES):
        sc = max(s[c], 8)
        while True:
            r = (off + sc) % BANK
            if r == 0 or 8 <= r <= 504:
                break
            sc += 1
        out.append(sc)
        off += sc
    return out


def _layout(y):
    s = _class_sizes(y)
    tot = sum(s)
    Lp = ((tot + BANK - 1) // BANK) * BANK
    offs = np.concatenate([[0], np.cumsum(s)])  # class windows [offs[c], offs[c+1])
    return s, offs, Lp


GROUP = 4  # banks per matmul group (2*GROUP <= RING_BANKS)


def _tail_gstarts(nbanks):
    """Matmul group starts for the LAST qtile: normal GROUP-wide groups,
    but the final full group is split 2+2 (plus the partial last bank) so
    the end-of-kernel scans overlap the matmul stream instead of trailing
    it."""
    t0 = ((nbanks - 1) // GROUP) * GROUP  # start of the partial tail group
    gs = list(range(0, max(t0 - GROUP, 0), GROUP))
    if t0 >= GROUP:
        gs += list(range(t0 - GROUP, t0))  # 1-bank groups for the last few
    gs += [t0]
    return gs


def _pieces_for_qtile(qt, offs, Lp, nbanks, last=False):
    """Scan pieces for one query tile: class windows split at ring wraps
    AND at matmul-group boundaries (so each piece's max8 only waits on its
    own group's last pass, releasing PSUM banks to the PE sooner).  The
    last qtile also splits at its finer tail-group boundaries.

    Tail padding past offs[-1] is never scanned (its scores are garbage-
    free anyway, but skipping it saves DVE cycles).

    Returns list of (col_off, length, class)."""
    x0 = (-nbanks * qt * BANK) % RING
    splits = set(range(x0, Lp + 1, RING)) | set(range(0, Lp + 1, GROUP * BANK))
    if last:
        splits |= {g * BANK for g in _tail_gstarts(nbanks)}
    pieces = []
    for c in range(NUM_CLASSES):
        o, e = int(offs[c]), int(offs[c + 1])
        cuts = sorted([o, e] + [x for x in splits if o < x < e])
        for a, b in zip(cuts[:-1], cuts[1:]):
            pieces.append((a, b - a, c))
    return pieces


def _build_program(NQ, Lp, offs):
    import concourse.bacc as bacc
    import concourse.tile as tile
    import concourse.mybir as mybir

    nbanks = Lp // BANK
    ctot = int(offs[-1])  # real content columns; the rest is tail padding
    nqt = NQ // QTILE

    def bank_w(b):
        # partial last bank: matmuls stream only the content columns
        return min(BANK, ctot - b * BANK)

    all_pieces = [_pieces_for_qtile(qt, offs, Lp, nbanks, last=(qt == nqt - 1))
                  for qt in range(nqt)]
    slot_base = np.cumsum([0] + [len(p) for p in all_pieces])
    STOT = int(slot_base[-1])

    f16 = mybir.dt.float16
    nc = bacc.Bacc("TRN2", target_bir_lowering=False, debug=False)
    qh = nc.dram_tensor("qh", [D, NQ], f16, kind="ExternalInput")
    s2 = nc.dram_tensor("s2", [D, NQ], f16, kind="ExternalInput")
    s3 = nc.dram_tensor("s3", [D, NQ], f16, kind="ExternalInput")
    # ma = x_h, shared by pass1 (qh) and pass2 (q_l): loaded from HBM once.
    # mb = [x_l rows 0-125; xx_h; xx_l] for pass3.
    ma = nc.dram_tensor("ma", [D, Lp], f16, kind="ExternalInput")
    mb = nc.dram_tensor("mb", [D, Lp], f16, kind="ExternalInput")
    out_cand = nc.dram_tensor("out_cand", [QTILE, STOT * 8], mybir.dt.float32,
                              kind="ExternalOutput")

    with tile.TileContext(nc) as tc:
        with tc.tile_pool(name="res", bufs=1) as res, \
             tc.tile_pool(name="ring", bufs=1, space="PSUM") as ringp:
            qh_t = res.tile([D, NQ], f16)
            s2_t = res.tile([D, NQ], f16)
            s3_t = res.tile([D, NQ], f16)
            ma_t = res.tile([D, Lp], f16)
            mb_t = res.tile([D, Lp], f16)
            cand_t = res.tile([QTILE, STOT * 8], mybir.dt.float32)
            # warm the PE's HAM clock gate while input DMAs are in flight:
            # junk matmuls on a zeroed SBUF tile run as soon as the (gpsimd)
            # memset lands, keeping the PE at full clock by the time real
            # work starts (values are overwritten by start=True).
            ring = ringp.tile([QTILE, RING], mybir.dt.float32)
            junk = res.tile([D, BANK], f16)
            nc.gpsimd.memset(junk[:], 0.0)

            # load order matters: get exactly what qtile 0 group 0 needs
            # first (qh slice + first 4 ma banks), then the rest in
            # consumption order, so real matmuls can start ~1.7us after
            # DMA kickoff instead of waiting for full first chunks.
            QSPLIT = 8
            qs = NQ // QSPLIT
            g0w = GROUP * BANK

            def qsl_i(i):
                return slice(i * qs, (i + 1) * qs)

            nc.sync.dma_start(out=qh_t[:, 0:QTILE], in_=qh[:, 0:QTILE])
            nc.sync.dma_start(out=ma_t[:, 0:g0w], in_=ma[:, 0:g0w])
            nc.sync.dma_start(out=s2_t[:, 0:QTILE], in_=s2[:, 0:QTILE])
            nc.sync.dma_start(out=mb_t[:, 0:g0w], in_=mb[:, 0:g0w])
            nc.sync.dma_start(out=s3_t[:, 0:QTILE], in_=s3[:, 0:QTILE])
            # all m chunks before the remaining q slices: qtile 0 consumes
            # every m column in its first ~16us, while qtile 1+ only needs
            # later q slices after that.
            mcuts = [g0w]
            while mcuts[-1] < ctot:
                mcuts.append(min(mcuts[-1] + 2048, ctot))
            for a, b in zip(mcuts[:-1], mcuts[1:]):
                if a < b:
                    for t, d in ((ma_t, ma), (mb_t, mb)):
                        nc.sync.dma_start(out=t[:, a:b], in_=d[:, a:b])
            for t, d in ((qh_t, qh), (s2_t, s2), (s3_t, s3)):
                nc.sync.dma_start(out=t[:, QTILE:qs], in_=d[:, QTILE:qs])
            for i in range(1, QSPLIT):
                for t, d in ((qh_t, qh), (s2_t, s2), (s3_t, s3)):
                    nc.sync.dma_start(out=t[:, qsl_i(i)], in_=d[:, qsl_i(i)])

            # 9 junk matmuls: keeps the PE busy from preamble end until
            # the first real inputs land (~12us), so the HAM SHORT window
            # trips to full clock with no idle gap.
            for w in range(9):
                nc.tensor.matmul(ring[:, (w % RING_BANKS) * BANK:
                                       (w % RING_BANKS) * BANK + BANK],
                                 lhsT=junk[:, 0:QTILE], rhs=junk[:],
                                 start=True, stop=True)

            for qt in range(nqt):
                qsl = slice(qt * QTILE, (qt + 1) * QTILE)
                pieces = all_pieces[qt]
                base = int(slot_base[qt])
                emitted = 0

                def emit_scans(done_cols, emitted):
                    while emitted < len(pieces):
                        o, ln, _c = pieces[emitted]
                        if o + ln > done_cols:
                            break
                        ro = (qt * nbanks * BANK + o) % RING
                        slot = (base + emitted) * 8
                        nc.vector.max(out=cand_t[:, slot:slot + 8],
                                      in_=ring[:, ro:ro + ln])
                        emitted += 1
                    return emitted

                if qt == nqt - 1:
                    gstarts = _tail_gstarts(nbanks)
                else:
                    gstarts = list(range(0, nbanks, GROUP))
                gends = gstarts[1:] + [nbanks]
                mid_flush_slot = None
                for g0, g1 in zip(gstarts, gends):
                    banks = range(g0, g1)

                    def rsl(b):
                        s = ((qt * nbanks + b) % RING_BANKS) * BANK
                        return slice(s, s + bank_w(b))

                    def csl(b):
                        return slice(b * BANK, b * BANK + bank_w(b))

                    # per-bank pass order: each bank's scores complete 2
                    # passes earlier than pass-major order, so its max8
                    # starts sooner and the PSUM bank frees sooner; pieces
                    # ending mid-group (class-window boundaries) emit
                    # right after their bank's last pass.  Exception: the
                    # first two groups of qtile 0 run pass-major, deferring
                    # their mb (pass3) needs ~1.7us while its DMA lands.
                    if qt == 0 and g0 < 2 * GROUP:
                        for b in banks:
                            nc.tensor.matmul(ring[:, rsl(b)], lhsT=qh_t[:, qsl],
                                             rhs=ma_t[:, csl(b)],
                                             start=True, stop=False)
                        for b in banks:
                            nc.tensor.matmul(ring[:, rsl(b)], lhsT=s2_t[:, qsl],
                                             rhs=ma_t[:, csl(b)],
                                             start=False, stop=False)
                        for b in banks:
                            nc.tensor.matmul(ring[:, rsl(b)], lhsT=s3_t[:, qsl],
                                             rhs=mb_t[:, csl(b)],
                                             start=False, stop=True)
                            emitted = emit_scans((b + 1) * BANK, emitted)
                    else:
                        for b in banks:
                            nc.tensor.matmul(ring[:, rsl(b)], lhsT=qh_t[:, qsl],
                                             rhs=ma_t[:, csl(b)],
                                             start=True, stop=False)
                            nc.tensor.matmul(ring[:, rsl(b)], lhsT=s2_t[:, qsl],
                                             rhs=ma_t[:, csl(b)],
                                             start=False, stop=False)
                            nc.tensor.matmul(ring[:, rsl(b)], lhsT=s3_t[:, qsl],
                                             rhs=mb_t[:, csl(b)],
                                             start=False, stop=True)
                            emitted = emit_scans((b + 1) * BANK, emitted)
                    # last qtile: flush the bulk of its slots mid-qtile so
                    # the end-of-kernel flush (which the epilogue waits on)
                    # is tiny
                    if qt == nqt - 1 and g1 >= nbanks - GROUP \
                            and mid_flush_slot is None and emitted > 0:
                        mid_flush_slot = base + emitted
                        nc.sync.dma_start(
                            out=out_cand[:, base * 8:mid_flush_slot * 8],
                            in_=cand_t[:, base * 8:mid_flush_slot * 8])

                # stream candidates out in tapered batches: big batches
                # early, single qtiles at the end so the final flush (and
                # its descriptor-generation latency) is tiny
                flush = {7: 0, 15: 8, 23: 16, 27: 24, 29: 28, 30: 30, 31: 31}
                if qt in flush:
                    lo8 = (mid_flush_slot * 8 if qt == nqt - 1
                           and mid_flush_slot is not None
                           else int(slot_base[flush[qt]]) * 8)
                    hi8 = int(slot_base[qt + 1]) * 8
                    if lo8 < hi8:
                        nc.sync.dma_start(out=out_cand[:, lo8:hi8],
                                          in_=cand_t[:, lo8:hi8])

    nc.compile()
    return nc, all_pieces, slot_base


def _get_program(NQ, Lp, offs):
    key = (NQ, Lp, tuple(int(o) for o in offs))
    if key not in _compiled_cache:
        _compiled_cache[key] = _build_program(NQ, Lp, offs)
    return _compiled_cache[key]


def prepare(X_train, y_train, X_test):
    X_train = np.ascontiguousarray(np.asarray(X_train, dtype=np.float32))
    X_test = np.ascontiguousarray(np.asarray(X_test, dtype=np.float32))
    y = np.asarray(y_train).astype(np.int64)
    N, Dd = X_train.shape
    NQ = X_test.shape[0]
    assert Dd == D

    s, offs, Lp = _layout(y)

    order = np.argsort(y, kind="stable")
    n = np.bincount(y, minlength=NUM_CLASSES)
    cstart = np.concatenate([[0], np.cumsum(n)])

    xx = np.einsum("ij,ij->i", X_train.astype(np.float64),
                   X_train.astype(np.float64)).astype(np.float32)

    # per-core fp16 hi/lo train splits, class-contiguous layout
    xh = np.zeros((NCORES, D, Lp), np.float16)
    xl = np.zeros((NCORES, D, Lp), np.float16)
    xxh = np.full((NCORES, Lp), PAD_XXH, np.float16)
    xxl = np.zeros((NCORES, Lp), np.float16)
    for c in range(NUM_CLASSES):
        rows = order[cstart[c]:cstart[c + 1]]
        sc = s[c]
        o = int(offs[c])
        for k in range(NCORES):
            sub = rows[k * sc:(k + 1) * sc]
            m = len(sub)
            if m:
                xt32 = X_train[sub].T  # [D, m] fp32
                h = xt32.astype(np.float16)
                xh[k, :, o:o + m] = h
                xl[k, :, o:o + m] = (xt32 - h.astype(np.float32)).astype(np.float16)
                nx = -xx[sub]
                nh = nx.astype(np.float16)
                xxh[k, o:o + m] = nh
                xxl[k, o:o + m] = (nx - nh.astype(np.float32)).astype(np.float16)

    # moving tensors: ma = xh (shared by pass1+pass2);
    # mb = xl with rows 126/127 := xxh/xxl
    mb = xl.copy()
    mb[:, D - 2, :] = xxh
    mb[:, D - 1, :] = xxl

    # stationary tensors: qh = fp16(2*X_test)^T (pass1); s2 = q_l rows
    # (pass2, full 128 dims); s3 = [q_h rows 0-125; 1; 1] (pass3)
    q32 = (2.0 * X_test).T.astype(np.float32)          # [D, NQ]
    qh = q32.astype(np.float16)
    ql = (q32 - qh.astype(np.float32)).astype(np.float16)
    s3m = qh.copy()
    s3m[D - 2, :] = np.float16(1.0)
    s3m[D - 1, :] = np.float16(1.0)

    nc, all_pieces, slot_base = _get_program(NQ, Lp, offs)

    in_maps = [{"qh": qh, "s2": ql, "s3": s3m,
                "ma": np.ascontiguousarray(xh[k]),
                "mb": np.ascontiguousarray(mb[k])}
               for k in range(NCORES)]

    slot_labels = np.concatenate(
        [np.repeat([c for (_o, _l, c) in pieces], 8) for pieces in all_pieces])
    meta = (all_pieces, slot_base, slot_labels, NQ)
    return nc, in_maps, meta


def merge(results, meta):
    all_pieces, slot_base, slot_labels, NQ = meta
    nqt = NQ // QTILE
    K = 16
    vals = np.stack([results[k]["out_cand"] for k in range(NCORES)], axis=0)
    preds = np.empty(NQ, np.int64)
    for qt in range(nqt):
        lo = int(slot_base[qt]) * 8
        hi = int(slot_base[qt + 1]) * 8
        v = vals[:, :, lo:hi]                        # [NCORES, QTILE, W]
        lab = slot_labels[lo:hi]
        v = np.moveaxis(v, 1, 0).reshape(QTILE, -1)  # [QTILE, NCORES*W]
        labs = np.tile(lab, NCORES)
        sel = np.argpartition(-v, K - 1, axis=1)[:, :K]
        top_lab = labs[sel]
        counts = np.zeros((QTILE, NUM_CLASSES), np.int64)
        for c in range(NUM_CLASSES):
            counts[:, c] = (top_lab == c).sum(1)
        preds[qt * QTILE:(qt + 1) * QTILE] = counts.argmax(1)
    return preds.astype(np.int64)


def kernel(X_train, y_train, X_test):
    from concourse.bass_utils import run_bass_kernel_spmd
    nc, in_maps, meta = prepare(X_train, y_train, X_test)
    res = run_bass_kernel_spmd(nc, in_maps, core_ids=list(range(NCORES)))
    return merge(res.results, meta)

